# revision 19
# baseline (speedup 1.0000x reference)
"""Trainium2 Bass kernel for the shifted-slice-copy stereo cost volume.

Reference semantics (B=2, C=32, H=128, W=240, D=max_disp//4=48):
    out[:, :C,  d, :, w] = left[:, :, :, w]      if w >= d else 0
    out[:, C:,  d, :, w] = right[:, :, :, w - d] if w >= d else 0
    out shape [B, 2C, D, H, W] float32  (~755 MB)

Memory-regime problem: per core ~94 MB of HBM writes dominate.  Measured
on these cores the SBUF AXI fabric (~435 GB/s) is the wall, not the
358 GB/s nominal HBM-per-NC share, so it pays to write the masked zeros
too and use maximal descriptors everywhere:

For each disparity d, compute engines materialize the full masked/
shifted [2*CPC, H, W] slab into an SBUF "page" ([128 part x 3840 f32],
partition = (half, c, 16-row strip)); the sync-engine HWDGE ring then
stores the page as ONE dma_start with 128 contiguous ~15 KB descriptors
(16 rows each) running at SDMA line rate, vs ~30x more ~900 B row
descriptors (~15 ns fixed cost each) for a valid-suffix-only scheme.

Keys (all hardware-measured):
  - descriptors are dealt to the 16 SDMA engines round-robin over the
    OUTERMOST dst AP dim -> keep it 16 (= 2*CPC slabs) or 128.
  - left page halves are initialized once and never copied again:
    reusing page b at step d only memsets the newly-invalid columns
    [d-NB, d) (the left data is d-independent, only the mask grows).
    Row j=0 of each strip is skipped by the store (flat suffix [d:]),
    so its stale prefix never needs zeroing.
  - right halves: per-d valid-only strided copy [p, j, d:W] <- [p, j,
    0:W-d], split 9/5/2 rows across DVE / ACT / GpSimd (measured
    relative copy speeds under DMA load), + incremental prefix memset.
  - per-dma_start completion semaphores must NOT be pooled across
    dma_starts: engines complete different DMAs out of order, so a
    pooled counter reaching 16*k does NOT mean the k-th DMA finished.
    One semaphore per load and per page slot.

Sharding: 8 cores = 2 batches x 4 channel-blocks of 8 channels; purely
data-parallel, no communication.
"""

import sys
from contextlib import ExitStack

import numpy as np

for _p in ("/opt/trn_rl_repo",):
    if _p not in sys.path:
        sys.path.insert(0, _p)

import concourse.bass as bass
from concourse import mybir
from concourse.bass_utils import run_bass_kernel_spmd

B, C, H, W = 2, 32, 128, 240
D = 48          # max_disp // 4
CPC = 8         # channels per core (C / 4 channel-blocks)
NCORES = 8
J = 16          # rows per strip
T = H // J      # strips per channel (8)
FREE = J * W    # f32 elements per partition per page (3840)
K = 48          # d < K: full-strip page stores; d >= K: per-row stores
NB = 8          # page buffers in flight
JV = 9          # DVE copies rows [0:JV)
JS = 14         # ACT copies rows [JV:JS); GpSimd rows [JS:J)

_NC_CACHE = None


def _build_bass():
    """One core's program: [CPC,H,W] left/right shard -> [2*CPC,D,H,W] out."""
    nc = bass.Bass()
    f32 = mybir.dt.float32
    left_c = nc.declare_dram_parameter("left_c", [CPC, H, W], f32, isOutput=False)
    right_c = nc.declare_dram_parameter("right_c", [CPC, H, W], f32, isOutput=False)
    out_c = nc.declare_dram_parameter("out_c", [2 * CPC, D, H, W], f32, isOutput=True)

    need_row = K < D

    with ExitStack() as ctx:
        S = ctx.enter_context(nc.sbuf_tensor("S", [128, FREE], f32))
        if need_row:
            Srow = ctx.enter_context(
                nc.sbuf_tensor("Srow", [128, 2 * CPC * W], f32)
            )
            loadRow_sem = ctx.enter_context(nc.semaphore("loadRow_sem"))
            sb_sem = ctx.enter_context(nc.semaphore("sb_sem"))
        P = ctx.enter_context(nc.sbuf_tensor("P", [128, NB * FREE], f32))
        loadL_sem = ctx.enter_context(nc.semaphore("loadL_sem"))
        loadR_sem = ctx.enter_context(nc.semaphore("loadR_sem"))
        bl_sem = ctx.enter_context(nc.semaphore("bl_sem"))
        bs_sem = ctx.enter_context(nc.semaphore("bs_sem"))
        bg_sem = ctx.enter_context(nc.semaphore("bg_sem"))
        # One completion semaphore per page slot: pooled counters are racy
        # (engines complete different dma_starts out of order).
        sa_slot = [
            ctx.enter_context(nc.semaphore(f"sa{b}_sem")) for b in range(NB)
        ]
        block = ctx.enter_context(nc.Block())

        # S partition p = half*64 + c*T + t holds rows 16t..16t+15 of that
        # channel, [j*W + w] in the free dim.
        S3 = S[:, :].rearrange("p (j w) -> p j w", j=J)

        def page(b):
            return P[:, b * FREE : (b + 1) * FREE]

        def page3(b):
            return page(b).rearrange("p (j w) -> p j w", j=J)

        def slot_uses(b):
            return (K - b + NB - 1) // NB  # how many steps use slot b

        # ---- sync: loads, then stream A (full-strip page stores) ----
        @block.sync
        def _(sync):
            sync.dma_start(
                S[0:64, :].rearrange("p (j w) -> p j w", j=J),
                left_c[:, :, :].rearrange("c (t j) w -> (c t) j w", j=J),
            ).then_inc(loadL_sem, 16)
            sync.dma_start(
                S[64:128, :].rearrange("p (j w) -> p j w", j=J),
                right_c[:, :, :].rearrange("c (t j) w -> (c t) j w", j=J),
            ).then_inc(loadR_sem, 16)
            if need_row:
                Srow3 = Srow[:, :].rearrange("p (cc w) -> p cc w", cc=2 * CPC)
                sync.dma_start(
                    Srow3[:, 0:CPC, :],
                    left_c[:, :, :].rearrange("c h w -> h c w"),
                ).then_inc(loadRow_sem, 16)
                sync.dma_start(
                    Srow3[:, CPC : 2 * CPC, :],
                    right_c[:, :, :].rearrange("c h w -> h c w"),
                ).then_inc(loadRow_sem, 16)
            for d in range(K):
                sync.wait_ge(bl_sem, d + 1)
                sync.wait_ge(bs_sem, d + 1)
                sync.wait_ge(bg_sem, d + 1)
                # out[:, d] full planes <- page(d%NB): 128 x ~15 KB descs
                # (skip the flat prefix [0:d) = row-0 masked zeros).
                sync.dma_start(
                    out_c[:, d, :, :].rearrange("cc (t j) w -> cc t (j w)", j=J)[
                        :, :, d:FREE
                    ],
                    page(d % NB)[:, d:FREE],
                ).then_inc(sa_slot[d % NB], 16)
            for b in range(min(NB, K)):
                sync.wait_ge(sa_slot[b], 16 * slot_uses(b))

        # ---- scalar: stream B row stores (if any), else 3rd builder ----
        @block.scalar
        def _(scalar):
            if need_row:
                Srow3 = Srow[:, :].rearrange("p (cc w) -> p cc w", cc=2 * CPC)
                scalar.wait_ge(loadRow_sem, 32)
                n = 0
                for d in range(K, D):
                    scalar.dma_start(
                        out_c[0:CPC, d, :, d:W].rearrange("c h w -> h c w"),
                        Srow3[:, 0:CPC, d:W],
                    ).then_inc(sb_sem, 16)
                    scalar.dma_start(
                        out_c[CPC : 2 * CPC, d, :, d:W].rearrange("c h w -> h c w"),
                        Srow3[:, CPC : 2 * CPC, 0 : W - d],
                    ).then_inc(sb_sem, 16)
                    n += 2
                if n:
                    scalar.wait_ge(sb_sem, 16 * n)
            scalar.wait_ge(loadR_sem, 16)
            for d in range(K):
                b = d % NB
                if d >= NB:
                    scalar.wait_ge(sa_slot[b], 16 * (d // NB))
                ins = scalar.copy(
                    page3(b)[64:128, JV:JS, d:W], S3[64:128, JV:JS, 0 : W - d]
                )
                ins.then_inc(bs_sem, 1)

        # ---- vector: page left-half init + mask upkeep + right copy lo ----
        # Right-half copies are strided valid-only ([p, j, d:W] <- [p, j,
        # 0:W-d]) so the masked prefix is never written by the copy; the
        # prefix (rows j>=1; row 0's prefix is never stored) is kept zero
        # incrementally: reusing buffer b at step d only zeros the
        # newly-invalid columns [d-NB, d).  First use (d < NB) zeros the
        # full [0, d) prefix, covering uninitialized SBUF.
        @block.vector
        def _(vector):
            vector.wait_ge(loadL_sem, 16)
            for b in range(min(NB, K)):
                vector.tensor_copy(page(b)[0:64, :], S[0:64, :])
            vector.wait_ge(loadR_sem, 16)
            for d in range(K):
                b = d % NB
                if d >= NB:
                    vector.wait_ge(sa_slot[b], 16 * (d // NB))
                lo = max(0, d - NB)
                if d > 0:
                    vector.memset(page3(b)[:, 1:J, lo:d], 0.0)
                ins = vector.tensor_copy(
                    page3(b)[64:128, 0:JV, d:W], S3[64:128, 0:JV, 0 : W - d]
                )
                ins.then_inc(bl_sem, 1)

        # ---- gpsimd: right copy hi rows ----
        @block.gpsimd
        def _(gpsimd):
            gpsimd.wait_ge(loadR_sem, 16)
            for d in range(K):
                b = d % NB
                if d >= NB:
                    gpsimd.wait_ge(sa_slot[b], 16 * (d // NB))
                ins = gpsimd.tensor_copy(
                    page3(b)[64:128, JS:J, d:W], S3[64:128, JS:J, 0 : W - d]
                )
                ins.then_inc(bg_sem, 1)

    return nc


def _get_nc():
    global _NC_CACHE
    if _NC_CACHE is None:
        _NC_CACHE = _build_bass()
    return _NC_CACHE


def _shard_inputs(left, right):
    in_maps = []
    for i in range(NCORES):
        b, blk = divmod(i, 4)
        c0 = blk * CPC
        in_maps.append(
            {
                "left_c": np.ascontiguousarray(left[b, c0 : c0 + CPC]),
                "right_c": np.ascontiguousarray(right[b, c0 : c0 + CPC]),
            }
        )
    return in_maps


def _gather_outputs(results):
    out = np.empty((B, 2 * C, D, H, W), np.float32)
    for i in range(NCORES):
        b, blk = divmod(i, 4)
        c0 = blk * CPC
        oc = results[i]["out_c"]
        out[b, c0 : c0 + CPC] = oc[:CPC]
        out[b, C + c0 : C + c0 + CPC] = oc[CPC:]
    return out


def run_sharded(left, right, **run_kwargs):
    """Compile+run the SPMD kernel; returns (full_output, BassKernelResults)."""
    res = run_bass_kernel_spmd(
        _get_nc(), _shard_inputs(left, right), list(range(NCORES)), **run_kwargs
    )
    return _gather_outputs(res.results), res


def kernel(**inputs):
    left = np.asarray(inputs["left_feature"], dtype=np.float32)
    right = np.asarray(inputs["right_feature"], dtype=np.float32)
    max_disp = int(np.asarray(inputs["max_disp"]))
    assert left.shape == (B, C, H, W), left.shape
    assert right.shape == (B, C, H, W), right.shape
    assert max_disp // 4 == D, max_disp
    out, _ = run_sharded(left, right)
    return out


# revision 20
# speedup vs baseline: 1.1560x; 1.1560x over previous
"""Trainium2 Bass kernel for the shifted-slice-copy stereo cost volume.

Reference semantics (B=2, C=32, H=128, W=240, D=max_disp//4=48):
    out[:, :C,  d, :, w] = left[:, :, :, w]      if w >= d else 0
    out[:, C:,  d, :, w] = right[:, :, :, w - d] if w >= d else 0
    out shape [B, 2C, D, H, W] float32  (~755 MB)

Memory-regime problem: per core ~94 MB of HBM writes dominate.  Measured
on these cores the SBUF AXI fabric (~435 GB/s) is the wall, not the
358 GB/s nominal HBM-per-NC share, so it pays to write the masked zeros
too and use maximal descriptors everywhere:

For each disparity d, compute engines materialize the full masked/
shifted [2*CPC, H, W] slab into an SBUF "page" ([128 part x 3840 f32],
partition = (half, c, 16-row strip)); the sync-engine HWDGE ring then
stores the page as ONE dma_start with 128 contiguous ~15 KB descriptors
(16 rows each) running at SDMA line rate, vs ~30x more ~900 B row
descriptors (~15 ns fixed cost each) for a valid-suffix-only scheme.

Keys (all hardware-measured):
  - descriptors are dealt to the 16 SDMA engines round-robin over the
    OUTERMOST dst AP dim -> keep it 16 (= 2*CPC slabs) or 128.
  - left page halves are initialized once and never copied again:
    reusing page b at step d only memsets the newly-invalid columns
    [d-NB, d) (the left data is d-independent, only the mask grows).
    Row j=0 of each strip is skipped by the store (flat suffix [d:]),
    so its stale prefix never needs zeroing.
  - right halves: per-d valid-only strided copy [p, j, d:W] <- [p, j,
    0:W-d], split 9/5/2 rows across DVE / ACT / GpSimd (measured
    relative copy speeds under DMA load), + incremental prefix memset.
  - per-dma_start completion semaphores must NOT be pooled across
    dma_starts: engines complete different DMAs out of order, so a
    pooled counter reaching 16*k does NOT mean the k-th DMA finished.
    One semaphore per load and per page slot.

Sharding: 8 cores = 2 batches x 4 channel-blocks of 8 channels; purely
data-parallel, no communication.
"""

import sys
from contextlib import ExitStack

import numpy as np

for _p in ("/opt/trn_rl_repo",):
    if _p not in sys.path:
        sys.path.insert(0, _p)

import concourse.bass as bass
from concourse import mybir
from concourse.bass_utils import run_bass_kernel_spmd

B, C, H, W = 2, 32, 128, 240
D = 48          # max_disp // 4
CPC = 8         # channels per core (C / 4 channel-blocks)
NCORES = 8
J = 16          # rows per strip
T = H // J      # strips per channel (8)
FREE = J * W    # f32 elements per partition per page (3840)
K = 20          # d < K: full-strip page stores; d >= K: per-row stores
NB = 6          # page buffers in flight
JV = 13         # DVE copies rows [0:JV)
JS = 13         # ACT copies rows [JV:JS); GpSimd rows [JS:J)

_NC_CACHE = None


def _build_bass():
    """One core's program: [CPC,H,W] left/right shard -> [2*CPC,D,H,W] out."""
    nc = bass.Bass()
    f32 = mybir.dt.float32
    left_c = nc.declare_dram_parameter("left_c", [CPC, H, W], f32, isOutput=False)
    right_c = nc.declare_dram_parameter("right_c", [CPC, H, W], f32, isOutput=False)
    out_c = nc.declare_dram_parameter("out_c", [2 * CPC, D, H, W], f32, isOutput=True)

    need_row = K < D

    with ExitStack() as ctx:
        S = ctx.enter_context(nc.sbuf_tensor("S", [128, FREE], f32))
        if need_row:
            Srow = ctx.enter_context(
                nc.sbuf_tensor("Srow", [128, 2 * CPC * W], f32)
            )
            loadRow_sem = ctx.enter_context(nc.semaphore("loadRow_sem"))
            sb_sem = ctx.enter_context(nc.semaphore("sb_sem"))
        P = ctx.enter_context(nc.sbuf_tensor("P", [128, NB * FREE], f32))
        loadL_sem = ctx.enter_context(nc.semaphore("loadL_sem"))
        loadR_sem = ctx.enter_context(nc.semaphore("loadR_sem"))
        bl_sem = ctx.enter_context(nc.semaphore("bl_sem"))
        bs_sem = ctx.enter_context(nc.semaphore("bs_sem"))
        bg_sem = ctx.enter_context(nc.semaphore("bg_sem"))
        # One completion semaphore per page slot: pooled counters are racy
        # (engines complete different dma_starts out of order).
        sa_slot = [
            ctx.enter_context(nc.semaphore(f"sa{b}_sem")) for b in range(NB)
        ]
        block = ctx.enter_context(nc.Block())

        # S partition p = half*64 + c*T + t holds rows 16t..16t+15 of that
        # channel, [j*W + w] in the free dim.
        S3 = S[:, :].rearrange("p (j w) -> p j w", j=J)

        def page(b):
            return P[:, b * FREE : (b + 1) * FREE]

        def page3(b):
            return page(b).rearrange("p (j w) -> p j w", j=J)

        def slot_uses(b):
            return (K - b + NB - 1) // NB  # how many steps use slot b

        # ---- sync: loads, then stream A (full-strip page stores) ----
        @block.sync
        def _(sync):
            sync.dma_start(
                S[0:64, :].rearrange("p (j w) -> p j w", j=J),
                left_c[:, :, :].rearrange("c (t j) w -> (c t) j w", j=J),
            ).then_inc(loadL_sem, 16)
            sync.dma_start(
                S[64:128, :].rearrange("p (j w) -> p j w", j=J),
                right_c[:, :, :].rearrange("c (t j) w -> (c t) j w", j=J),
            ).then_inc(loadR_sem, 16)
            if need_row:
                Srow3 = Srow[:, :].rearrange("p (cc w) -> p cc w", cc=2 * CPC)
                sync.dma_start(
                    Srow3[:, 0:CPC, :],
                    left_c[:, :, :].rearrange("c h w -> h c w"),
                ).then_inc(loadRow_sem, 16)
                sync.dma_start(
                    Srow3[:, CPC : 2 * CPC, :],
                    right_c[:, :, :].rearrange("c h w -> h c w"),
                ).then_inc(loadRow_sem, 16)
            for d in range(K):
                sync.wait_ge(bl_sem, d + 1)
                if JS > JV:
                    sync.wait_ge(bs_sem, d + 1)
                sync.wait_ge(bg_sem, d + 1)
                # out[:, d] full planes <- page(d%NB): 128 x ~15 KB descs
                # (skip the flat prefix [0:d) = row-0 masked zeros).
                sync.dma_start(
                    out_c[:, d, :, :].rearrange("cc (t j) w -> cc t (j w)", j=J)[
                        :, :, d:FREE
                    ],
                    page(d % NB)[:, d:FREE],
                ).then_inc(sa_slot[d % NB], 16)
            for b in range(min(NB, K)):
                sync.wait_ge(sa_slot[b], 16 * slot_uses(b))

        # ---- scalar: stream B row stores (if any), else 3rd builder ----
        @block.scalar
        def _(scalar):
            if need_row:
                Srow3 = Srow[:, :].rearrange("p (cc w) -> p cc w", cc=2 * CPC)
                scalar.wait_ge(loadRow_sem, 32)
                n = 0
                for d in range(K, D):
                    scalar.dma_start(
                        out_c[0:CPC, d, :, d:W].rearrange("c h w -> h c w"),
                        Srow3[:, 0:CPC, d:W],
                    ).then_inc(sb_sem, 16)
                    scalar.dma_start(
                        out_c[CPC : 2 * CPC, d, :, d:W].rearrange("c h w -> h c w"),
                        Srow3[:, CPC : 2 * CPC, 0 : W - d],
                    ).then_inc(sb_sem, 16)
                    n += 2
                if n:
                    scalar.wait_ge(sb_sem, 16 * n)
            if JS > JV:
                scalar.wait_ge(loadR_sem, 16)
                for d in range(K):
                    b = d % NB
                    if d >= NB:
                        scalar.wait_ge(sa_slot[b], 16 * (d // NB))
                    ins = scalar.copy(
                        page3(b)[64:128, JV:JS, d:W], S3[64:128, JV:JS, 0 : W - d]
                    )
                    ins.then_inc(bs_sem, 1)

        # ---- vector: page left-half init + mask upkeep + right copy lo ----
        # Right-half copies are strided valid-only ([p, j, d:W] <- [p, j,
        # 0:W-d]) so the masked prefix is never written by the copy; the
        # prefix (rows j>=1; row 0's prefix is never stored) is kept zero
        # incrementally: reusing buffer b at step d only zeros the
        # newly-invalid columns [d-NB, d).  First use (d < NB) zeros the
        # full [0, d) prefix, covering uninitialized SBUF.
        @block.vector
        def _(vector):
            vector.wait_ge(loadL_sem, 16)
            for b in range(min(NB, K)):
                vector.tensor_copy(page(b)[0:64, :], S[0:64, :])
            vector.wait_ge(loadR_sem, 16)
            for d in range(K):
                b = d % NB
                if d >= NB:
                    vector.wait_ge(sa_slot[b], 16 * (d // NB))
                lo = max(0, d - NB)
                if d > 0:
                    vector.memset(page3(b)[:, 1:J, lo:d], 0.0)
                ins = vector.tensor_copy(
                    page3(b)[64:128, 0:JV, d:W], S3[64:128, 0:JV, 0 : W - d]
                )
                ins.then_inc(bl_sem, 1)

        # ---- gpsimd: right copy hi rows ----
        @block.gpsimd
        def _(gpsimd):
            gpsimd.wait_ge(loadR_sem, 16)
            for d in range(K):
                b = d % NB
                if d >= NB:
                    gpsimd.wait_ge(sa_slot[b], 16 * (d // NB))
                ins = gpsimd.tensor_copy(
                    page3(b)[64:128, JS:J, d:W], S3[64:128, JS:J, 0 : W - d]
                )
                ins.then_inc(bg_sem, 1)

    return nc


def _get_nc():
    global _NC_CACHE
    if _NC_CACHE is None:
        _NC_CACHE = _build_bass()
    return _NC_CACHE


def _shard_inputs(left, right):
    in_maps = []
    for i in range(NCORES):
        b, blk = divmod(i, 4)
        c0 = blk * CPC
        in_maps.append(
            {
                "left_c": np.ascontiguousarray(left[b, c0 : c0 + CPC]),
                "right_c": np.ascontiguousarray(right[b, c0 : c0 + CPC]),
            }
        )
    return in_maps


def _gather_outputs(results):
    out = np.empty((B, 2 * C, D, H, W), np.float32)
    for i in range(NCORES):
        b, blk = divmod(i, 4)
        c0 = blk * CPC
        oc = results[i]["out_c"]
        out[b, c0 : c0 + CPC] = oc[:CPC]
        out[b, C + c0 : C + c0 + CPC] = oc[CPC:]
    return out


def run_sharded(left, right, **run_kwargs):
    """Compile+run the SPMD kernel; returns (full_output, BassKernelResults)."""
    res = run_bass_kernel_spmd(
        _get_nc(), _shard_inputs(left, right), list(range(NCORES)), **run_kwargs
    )
    return _gather_outputs(res.results), res


def kernel(**inputs):
    left = np.asarray(inputs["left_feature"], dtype=np.float32)
    right = np.asarray(inputs["right_feature"], dtype=np.float32)
    max_disp = int(np.asarray(inputs["max_disp"]))
    assert left.shape == (B, C, H, W), left.shape
    assert right.shape == (B, C, H, W), right.shape
    assert max_disp // 4 == D, max_disp
    out, _ = run_sharded(left, right)
    return out
